# revision 1
# baseline (speedup 1.0000x reference)
# Trainium2 Bass kernel for nn_LocalEncoder (4-block local-attention encoder).
#
# Sharding: data-parallel over batch. Core c processes batch element c
# (B=8 == n_cores=8). Same SPMD program on every core, different x slice.
#
# Per-core dataflow: residual x [4096, 256] fp32 lives in SBUF for all 4
# blocks; block weights are DMA'd per block (double buffered); attention is
# computed windowed (128-token windows, look-around of +-1 window) with the
# score matrix built TRANSPOSED (keys on partitions) so A^T feeds the A@V
# matmul directly; softmax denominators come from ones-matmuls on the PE and
# are broadcast back over head rows with a selector matmul.

import numpy as np
import ml_dtypes

import concourse.bass as bass
import concourse.tile as tile
from concourse import bacc, mybir
from concourse.bass_utils import run_bass_kernel_spmd

F32 = mybir.dt.float32
BF16 = mybir.dt.bfloat16
NPBF = ml_dtypes.bfloat16

B, N, D = 8, 4096, 256
H, DH, WIN = 8, 32, 128
NW = N // WIN            # 32 windows
NB = 4                   # encoder blocks
FFI = 682                # geglu inner
FFP = 768                # padded inner (6 k-tiles of 128)
SCALE = DH ** -0.5
T512 = 512               # token tile for dense matmuls
NT = N // T512           # 8 token tiles
EPS = 1e-5
DEBUG = False            # emit block-0 intermediate taps + stop after block 0


# ---------------------------------------------------------------- host prep
def _prep_block_weights(i, ln1_g, ln1_b, qkv_w, out_w, ln2_g, ln2_b, ff_w1, ff_w2):
    """Fold LN gamma/beta + softmax scale into weights; pad FF; cast bf16."""
    g1, b1 = ln1_g[i].astype(np.float64), ln1_b[i].astype(np.float64)
    g2, b2 = ln2_g[i].astype(np.float64), ln2_b[i].astype(np.float64)
    Wqkv = qkv_w[i].astype(np.float64)          # [768, 256] (e, d)
    Wg = Wqkv * g1[None, :]
    bias_qkv = Wqkv @ b1                        # [768]
    # fold softmax scale into Q rows
    Wg[:256] *= SCALE
    bias_qkv = bias_qkv.copy()
    bias_qkv[:256] *= SCALE
    wqkT = np.ascontiguousarray(Wg[:512].T)     # [256, 512]
    bqk = bias_qkv[:512]                        # [512]
    wvT = np.ascontiguousarray(Wg[512:768].T)   # [256, 256]
    assert np.allclose(bias_qkv[512:], 0.0), "nonzero V bias unsupported"
    woT = np.ascontiguousarray(out_w[i].astype(np.float64).T)  # [256 e, 256 d]

    W1 = ff_w1[i].astype(np.float64) * g2[None, :]   # [1364, 256]
    b1f = ff_w1[i].astype(np.float64) @ b2           # [1364]
    a_part, g_part = W1[:FFI], W1[FFI:]
    ba, bg = b1f[:FFI], b1f[FFI:]
    assert np.allclose(ba, 0.0), "nonzero FF a-bias unsupported"
    aP = np.zeros((FFP, 256)); aP[:FFI] = a_part
    gP = np.zeros((FFP, 256)); gP[:FFI] = g_part
    bgP = np.zeros((FFP,)); bgP[:FFI] = bg
    w1aT = np.ascontiguousarray(aP.T)            # [256, 768]
    w1gT = np.ascontiguousarray(gP.T)            # [256, 768]
    W2 = np.zeros((FFP, 256)); W2[:FFI] = ff_w2[i].astype(np.float64).T
    w2T = np.ascontiguousarray(W2)               # [768, 256]

    c = lambda a: np.ascontiguousarray(a).astype(NPBF)
    return {
        f"wqkT_{i}": c(wqkT), f"bqk_{i}": np.ascontiguousarray(bqk).astype(np.float32),
        f"wvT_{i}": c(wvT), f"woT_{i}": c(woT),
        f"w1aT_{i}": c(w1aT), f"w1gT_{i}": c(w1gT),
        f"bg_{i}": np.ascontiguousarray(bgP).astype(np.float32),
        f"w2T_{i}": c(w2T),
    }


def _consts():
    ident = np.eye(128, dtype=NPBF)
    ones = np.ones((128, 32), dtype=NPBF)
    return {"ident": ident, "ones1": ones}


# ---------------------------------------------------------------- device IR
def _build(nc):
    """Emit the whole 4-block encoder as one Tile program."""
    x_d = nc.dram_tensor("x", (N, D), F32, kind="ExternalInput").ap()
    out_d = nc.dram_tensor("out", (N, D), F32, kind="ExternalOutput").ap()
    ident_d = nc.dram_tensor("ident", (128, 128), BF16, kind="ExternalInput").ap()
    ones_d = nc.dram_tensor("ones1", (128, 32), BF16, kind="ExternalInput").ap()
    wd = {}
    for i in range(NB):
        wd[f"wqkT_{i}"] = nc.dram_tensor(f"wqkT_{i}", (256, 512), BF16, kind="ExternalInput").ap()
        wd[f"bqk_{i}"] = nc.dram_tensor(f"bqk_{i}", (512,), F32, kind="ExternalInput").ap()
        wd[f"wvT_{i}"] = nc.dram_tensor(f"wvT_{i}", (256, 256), BF16, kind="ExternalInput").ap()
        wd[f"woT_{i}"] = nc.dram_tensor(f"woT_{i}", (256, 256), BF16, kind="ExternalInput").ap()
        wd[f"w1aT_{i}"] = nc.dram_tensor(f"w1aT_{i}", (256, FFP), BF16, kind="ExternalInput").ap()
        wd[f"w1gT_{i}"] = nc.dram_tensor(f"w1gT_{i}", (256, FFP), BF16, kind="ExternalInput").ap()
        wd[f"bg_{i}"] = nc.dram_tensor(f"bg_{i}", (FFP,), F32, kind="ExternalInput").ap()
        wd[f"w2T_{i}"] = nc.dram_tensor(f"w2T_{i}", (FFP, 256), BF16, kind="ExternalInput").ap()

    with tile.TileContext(nc) as tc:
        _emit(tc, x_d, out_d, ident_d, ones_d, wd)
    return nc


def _emit(tc, x_d, out_d, ident_d, ones_d, wd):
    nc = tc.nc
    from contextlib import ExitStack
    ctx = ExitStack()
    with ctx:
        consts = ctx.enter_context(tc.tile_pool(name="consts", bufs=1))
        resid = ctx.enter_context(tc.tile_pool(name="resid", bufs=1))
        seqbuf = ctx.enter_context(tc.tile_pool(name="seqbuf", bufs=1))
        wpool = ctx.enter_context(tc.tile_pool(name="wpool", bufs=2))

        ident = consts.tile([128, 128], BF16)
        nc.sync.dma_start(out=ident, in_=ident_d)
        ones1 = consts.tile([128, 32], BF16)
        nc.sync.dma_start(out=ones1, in_=ones_d)
        epsT = consts.tile([128, 1], F32)
        nc.vector.memset(epsT, EPS)

        # residual x, token-major: [128 tok-in-window, 32 windows, 256]
        x_sb = resid.tile([128, NW, D], F32)
        x_wpd = x_d.rearrange("(w p) d -> p w d", p=WIN)
        for c in range(8):
            nc.sync.dma_start(out=x_sb[:, 4 * c:4 * c + 4, :], in_=x_wpd[:, 4 * c:4 * c + 4, :])

        # whole-sequence activation buffers
        qT = seqbuf.tile([128, 2, N], BF16)       # Q^T  rows: g half, (hh*32+dh)
        kT = seqbuf.tile([128, 2, N], BF16)       # K^T
        v_sb = seqbuf.tile([128, NW, H, DH], BF16)  # V token-major
        at_sb = seqbuf.tile([128, 4, H, 3 * WIN], BF16)  # A^T ring (4 slots)

        for blk in range(NB):
            wqk = wpool.tile([128, 2, 512], BF16)
            nc.sync.dma_start(out=wqk, in_=wd[f"wqkT_{blk}"].rearrange("(k p) e -> p k e", p=128))
            bqk = wpool.tile([128, 4], F32)
            nc.sync.dma_start(out=bqk, in_=wd[f"bqk_{blk}"].rearrange("(e p) -> p e", p=128))
            wv = wpool.tile([128, 2, 256], BF16)
            nc.sync.dma_start(out=wv, in_=wd[f"wvT_{blk}"].rearrange("(k p) e -> p k e", p=128))
            wo = wpool.tile([128, 2, 256], BF16)
            nc.sync.dma_start(out=wo, in_=wd[f"woT_{blk}"].rearrange("(k p) e -> p k e", p=128))
            w1a = wpool.tile([128, 2, FFP], BF16)
            nc.sync.dma_start(out=w1a, in_=wd[f"w1aT_{blk}"].rearrange("(k p) e -> p k e", p=128))
            w1g = wpool.tile([128, 2, FFP], BF16)
            nc.sync.dma_start(out=w1g, in_=wd[f"w1gT_{blk}"].rearrange("(k p) e -> p k e", p=128))
            bgt = wpool.tile([128, 6], F32)
            nc.sync.dma_start(out=bgt, in_=wd[f"bg_{blk}"].rearrange("(e p) -> p e", p=128))
            w2 = wpool.tile([128, 6, 256], BF16)
            nc.sync.dma_start(out=w2, in_=wd[f"w2T_{blk}"].rearrange("(k p) d -> p k d", p=128))

            _phase_qkv(tc, ctx, x_sb, qT, kT, v_sb, wqk, bqk, wv, ident, epsT)
            if DEBUG and blk == 0:
                for nm, tl in (("dbg_qT", qT), ("dbg_kT", kT)):
                    d = nc.dram_tensor(nm, (128, 2, N), BF16, kind="ExternalOutput").ap()
                    nc.sync.dma_start(out=d, in_=tl)
                dv = nc.dram_tensor("dbg_v", (128, NW, H, DH), BF16, kind="ExternalOutput").ap()
                nc.sync.dma_start(out=dv, in_=v_sb)
            _phase_attn(tc, ctx, x_sb, qT, kT, v_sb, at_sb, wo, ones1)
            if DEBUG and blk == 0:
                da = nc.dram_tensor("dbg_xattn", (128, NW, D), F32, kind="ExternalOutput").ap()
                nc.sync.dma_start(out=da, in_=x_sb)
                dat = nc.dram_tensor("dbg_at", (128, 4, H, 3 * WIN), BF16, kind="ExternalOutput").ap()
                nc.sync.dma_start(out=dat, in_=at_sb)
                break
            _phase_ff(tc, ctx, x_sb, w1a, w1g, bgt, w2, ident, epsT)

        out_wpd = out_d.rearrange("(w p) d -> p w d", p=WIN)
        for c in range(8):
            nc.sync.dma_start(out=out_wpd[:, 4 * c:4 * c + 4, :], in_=x_sb[:, 4 * c:4 * c + 4, :])


def _layernorm_t512(tc, pools, x_sb, t, ident, epsT, xhT):
    """LN over one 512-token tile -> transposed bf16 xhat [128, 2, 512]."""
    nc = tc.nc
    stat, ptrans = pools
    st = stat.tile([128, 4, 6], F32)
    mv = stat.tile([128, 4, 2], F32)
    rs = stat.tile([128, 4], F32)
    for q in range(4):
        w = 4 * t + q
        nc.vector.bn_stats(out=st[:, q, :], in_=x_sb[:, w, :])
        nc.vector.bn_aggr(out=mv[:, q, :], in_=st[:, q, :])
    # rs = exp(-0.5 * ln(var + eps))  (stays inside the exp table set)
    lnv = stat.tile([128, 4], F32)
    nc.scalar.activation(out=lnv, in_=mv[:, :, 1], func=mybir.ActivationFunctionType.Ln,
                         bias=epsT, scale=1.0)
    nc.scalar.activation(out=rs, in_=lnv, func=mybir.ActivationFunctionType.Exp,
                         bias=0.0, scale=-0.5)
    for q in range(4):
        w = 4 * t + q
        xh = stat.tile([128, D], BF16, tag="xh")
        # sbuf-only op: run on the otherwise-idle GpSimd engine
        nc.gpsimd.tensor_scalar(out=xh, in0=x_sb[:, w, :],
                                scalar1=mv[:, q, 0:1], scalar2=rs[:, q:q + 1],
                                op0=mybir.AluOpType.subtract, op1=mybir.AluOpType.mult)
        for dt in range(2):
            pt = ptrans.tile([128, 128], BF16, space="PSUM")
            nc.tensor.transpose(pt, xh[:, 128 * dt:128 * dt + 128], ident)
            nc.vector.tensor_copy(out=xhT[:, dt, 128 * q:128 * q + 128], in_=pt)


def _phase_qkv(tc, ctx, x_sb, qT, kT, v_sb, wqk, bqk, wv, ident, epsT):
    nc = tc.nc
    from contextlib import ExitStack
    with ExitStack() as pctx:
        stat = pctx.enter_context(tc.tile_pool(name="stat", bufs=3))
        xhp = pctx.enter_context(tc.tile_pool(name="xhp", bufs=2))
        ptrans = pctx.enter_context(tc.tile_pool(name="ptrans", bufs=2, space="PSUM"))
        mm = pctx.enter_context(tc.tile_pool(name="mmqkv", bufs=3, space="PSUM"))

        for t in range(NT):
            xhT = xhp.tile([128, 2, T512], BF16)
            _layernorm_t512(tc, (stat, ptrans), x_sb, t, ident, epsT, xhT)
            # Q^T / K^T : feature-major [e-tile 128, 512 tok]
            for et in range(4):
                ps = mm.tile([128, T512], F32, space="PSUM")
                for kt in range(2):
                    nc.tensor.matmul(ps, lhsT=wqk[:, kt, 128 * et:128 * et + 128],
                                     rhs=xhT[:, kt, :], start=(kt == 0), stop=(kt == 1))
                dst = qT if et < 2 else kT
                g = et % 2
                nc.vector.tensor_scalar(out=dst[:, g, T512 * t:T512 * (t + 1)], in0=ps,
                                        scalar1=bqk[:, et:et + 1], scalar2=None,
                                        op0=mybir.AluOpType.add)
            # V token-major
            for q in range(4):
                w = 4 * t + q
                psv = mm.tile([128, D], F32, space="PSUM", tag="psv", bufs=2)
                for kt in range(2):
                    nc.tensor.matmul(psv, lhsT=xhT[:, kt, 128 * q:128 * q + 128],
                                     rhs=wv[:, kt, :], start=(kt == 0), stop=(kt == 1))
                nc.vector.tensor_copy(out=v_sb[:, w, :, :].rearrange("p h e -> p (h e)"), in_=psv)


def _phase_attn(tc, ctx, x_sb, qT, kT, v_sb, at_sb, wo, ones1):
    nc = tc.nc
    from contextlib import ExitStack
    with ExitStack() as pctx:
        simp = pctx.enter_context(tc.tile_pool(name="simp", bufs=2, space="PSUM"))
        avp = pctx.enter_context(tc.tile_pool(name="avp", bufs=2, space="PSUM"))
        denp = pctx.enter_context(tc.tile_pool(name="denp", bufs=2, space="PSUM"))
        osbp = pctx.enter_context(tc.tile_pool(name="osbp", bufs=3))

        for step in range(NW + 2):
            if step < NW:
                _attn_scores(tc, simp, qT, kT, at_sb, step)
            w = step - 2
            if w >= 0:
                _attn_av(tc, (avp, denp, osbp), x_sb, v_sb, at_sb, wo, ones1, w)


def _attn_scores(tc, simp, qT, kT, at_sb, wp):
    """Block-column pass wp: simT[j in wp, q in wp-1..wp+1] for all heads + exp."""
    nc = tc.nc
    qlo = max(0, wp - 1) * WIN
    qhi = min(NW, wp + 2) * WIN
    qn = qhi - qlo
    aoff = qlo - (wp - 1) * WIN     # column offset inside the 384-wide ring slot
    slot = wp % 4
    for g in range(2):
        for pair in range(2):
            sq = simp.tile([128, 1024], F32, space="PSUM", tag="sim")
            for sub in range(2):
                hh = 2 * pair + sub
                nc.tensor.matmul(
                    sq[:, 512 * sub:512 * sub + qn],
                    lhsT=kT[32 * hh:32 * hh + 32, g, WIN * wp:WIN * (wp + 1)],
                    rhs=qT[32 * hh:32 * hh + 32, g, qlo:qhi],
                    start=True, stop=True, tile_position=(32 * hh, 0))
            src = sq.rearrange("p (s c) -> p s c", c=512)[:, :, 0:qn]
            dst = at_sb[:, slot, 4 * g + 2 * pair:4 * g + 2 * pair + 2, aoff:aoff + qn]
            nc.scalar.activation(out=dst, in_=src, func=mybir.ActivationFunctionType.Exp)


def _attn_av(tc, pools, x_sb, v_sb, at_sb, wo, ones1, w):
    """o_un = A^T-weighted V, denominators, normalize, out-proj, residual."""
    nc = tc.nc
    avp, denp, osbp = pools
    wks = [wk for wk in (w - 1, w, w + 1) if 0 <= wk < NW]
    av = avp.tile([128, 256], F32, space="PSUM", tag="av")
    den = denp.tile([128, 256], F32, space="PSUM", tag="den")
    # Loop order: head-group outer, key-window inner. Each col-group's psum
    # accumulation chain completes before the next group's start=True, which
    # is required under both the per-partition (sim) and whole-bank (hw)
    # has_written-clear models.
    for hh in range(4):
        for g in range(2):
            h = 4 * g + hh
            for jt, wk in enumerate(wks):
                slot = wk % 4
                qoff = (w - (wk - 1)) * WIN
                first = g == 0 and jt == 0
                last = jt == len(wks) - 1
                rhs_at = at_sb[:, slot, h, qoff:qoff + WIN]
                nc.tensor.matmul(av[32 * hh:32 * hh + 32, 128 * g:128 * g + 128],
                                 lhsT=v_sb[:, wk, h, :], rhs=rhs_at,
                                 start=first, stop=last, skip_group_check=True,
                                 tile_position=(0, 32 * hh))
    # Denominators: one N=256 matmul per (hh, jt) covers heads hh and hh+4
    # via a strided rhs over the g dim. ones [128, 32] stationary replicates
    # each den over 32 rows -> row-aligned with av for the normalize mul.
    atv = at_sb.rearrange("p s (g hh) q -> p s hh g q", g=2, hh=4)
    for hh in range(4):
        for jt, wk in enumerate(wks):
            slot = wk % 4
            qoff = (w - (wk - 1)) * WIN
            nc.tensor.matmul(den[32 * hh:32 * hh + 32, 0:256],
                             lhsT=ones1, rhs=atv[:, slot, hh, :, qoff:qoff + WIN],
                             start=(jt == 0), stop=(jt == len(wks) - 1),
                             skip_group_check=True, tile_position=(0, 32 * hh))
    # reciprocal via exp(-ln(x)) on ACT (same table set as the attention exp;
    # DVE's iterative reciprocal is ~2.5x slower)
    lnden = osbp.tile([128, 256], F32, tag="lnden")
    nc.scalar.activation(out=lnden, in_=den, func=mybir.ActivationFunctionType.Ln)
    rden = osbp.tile([128, 256], F32, tag="rden")
    nc.scalar.activation(out=rden, in_=lnden, func=mybir.ActivationFunctionType.Exp,
                         bias=0.0, scale=-1.0)
    if DEBUG and w == 5:
        davt = osbp.tile([128, 256], F32, tag="davt")
        nc.vector.tensor_copy(out=davt, in_=av)
        dav = nc.dram_tensor("dbg_av", (128, 256), F32, kind="ExternalOutput").ap()
        nc.sync.dma_start(out=dav, in_=davt)
        drd = nc.dram_tensor("dbg_rden", (128, 256), F32, kind="ExternalOutput").ap()
        nc.sync.dma_start(out=drd, in_=rden)
    dp = den  # den bank is dead after the reciprocal; reuse for out-proj delta
    for g in range(2):
        osb = osbp.tile([128, 128], BF16, tag="osb")
        nc.vector.tensor_tensor(out=osb, in0=av[:, 128 * g:128 * (g + 1)],
                                in1=rden[:, 128 * g:128 * (g + 1)],
                                op=mybir.AluOpType.mult)
        if DEBUG and w == 5:
            dos = nc.dram_tensor(f"dbg_osb{g}", (128, 128), BF16, kind="ExternalOutput").ap()
            nc.sync.dma_start(out=dos, in_=osb)
        nc.tensor.matmul(dp, lhsT=osb, rhs=wo[:, g, :], start=(g == 0), stop=(g == 1))
    if DEBUG and w == 5:
        ddpt = osbp.tile([128, 256], F32, tag="ddpt")
        nc.vector.tensor_copy(out=ddpt, in_=dp)
        ddp = nc.dram_tensor("dbg_dp", (128, 256), F32, kind="ExternalOutput").ap()
        nc.sync.dma_start(out=ddp, in_=ddpt)
    nc.vector.tensor_tensor(out=x_sb[:, w, :], in0=dp, in1=x_sb[:, w, :],
                            op=mybir.AluOpType.add)


def _phase_ff(tc, ctx, x_sb, w1a, w1g, bgt, w2, ident, epsT):
    nc = tc.nc
    from contextlib import ExitStack
    with ExitStack() as pctx:
        stat = pctx.enter_context(tc.tile_pool(name="statf", bufs=3))
        xhp = pctx.enter_context(tc.tile_pool(name="xhpf", bufs=1))
        ptrans = pctx.enter_context(tc.tile_pool(name="ptransf", bufs=2, space="PSUM"))
        mmf = pctx.enter_context(tc.tile_pool(name="mmf", bufs=2, space="PSUM"))
        ysp = pctx.enter_context(tc.tile_pool(name="ysp", bufs=2))
        glp = pctx.enter_context(tc.tile_pool(name="glp", bufs=2))

        # LN pass first for ALL tokens, then the gelu-heavy FF pass: keeps the
        # Ln/Exp and Gelu ACT table sets from alternating (2.7us per switch).
        xh2 = xhp.tile([128, 2, N], BF16)
        for t in range(NT):
            _layernorm_t512(tc, (stat, ptrans), x_sb, t, ident, epsT,
                            xh2.rearrange("p k (t c) -> p k t c", c=T512)[:, :, t, :])
        for t in range(NT):
            xhT = xh2[:, :, T512 * t:T512 * (t + 1)]
            ysb = ysp.tile([128, 6, T512], BF16)
            for i in range(6):
                psA = mmf.tile([128, T512], F32, space="PSUM", tag="psA")
                psG = mmf.tile([128, T512], F32, space="PSUM", tag="psG")
                for kt in range(2):
                    nc.tensor.matmul(psA, lhsT=w1a[:, kt, 128 * i:128 * (i + 1)],
                                     rhs=xhT[:, kt, :], start=(kt == 0), stop=(kt == 1))
                for kt in range(2):
                    nc.tensor.matmul(psG, lhsT=w1g[:, kt, 128 * i:128 * (i + 1)],
                                     rhs=xhT[:, kt, :], start=(kt == 0), stop=(kt == 1))
                gl = glp.tile([128, T512], BF16, tag="gl")
                nc.scalar.activation(out=gl, in_=psG, func=mybir.ActivationFunctionType.Gelu,
                                     bias=bgt[:, i:i + 1], scale=1.0)
                nc.vector.tensor_tensor(out=ysb[:, i, :], in0=psA, in1=gl,
                                        op=mybir.AluOpType.mult)
            for q in range(4):
                w = 4 * t + q
                dp2 = mmf.tile([128, D], F32, space="PSUM", tag="dp2")
                for kt in range(6):
                    nc.tensor.matmul(dp2, lhsT=ysb[:, kt, 128 * q:128 * (q + 1)],
                                     rhs=w2[:, kt, :], start=(kt == 0), stop=(kt == 5))
                nc.vector.tensor_tensor(out=x_sb[:, w, :], in0=dp2, in1=x_sb[:, w, :],
                                        op=mybir.AluOpType.add)


# ---------------------------------------------------------------- entry
_CACHE = {}


def _get_nc():
    if "nc" not in _CACHE:
        nc = bacc.Bacc("TRN2", target_bir_lowering=False, debug=False,
                       enable_asserts=False, num_devices=8)
        _build(nc)
        nc.compile()
        _CACHE["nc"] = nc
    return _CACHE["nc"]


def kernel(x, mask, ln1_g, ln1_b, qkv_w, out_w, ln2_g, ln2_b, ff_w1, ff_w2,
           _trace=False, **kw):
    assert x.shape == (B, N, D)
    nc = _get_nc()
    wmaps = _consts()
    for i in range(NB):
        wmaps.update(_prep_block_weights(i, ln1_g, ln1_b, qkv_w, out_w,
                                         ln2_g, ln2_b, ff_w1, ff_w2))
    in_maps = []
    for c in range(B):
        m = dict(wmaps)
        m["x"] = np.ascontiguousarray(x[c]).astype(np.float32)
        in_maps.append(m)
    res = run_bass_kernel_spmd(nc, in_maps, core_ids=list(range(8)), trace=_trace)
    out = np.stack([res.results[c]["out"] for c in range(B)], axis=0)
    if _trace:
        return out.astype(np.float32), res
    return out.astype(np.float32)



# revision 3
# speedup vs baseline: 2.1039x; 2.1039x over previous
# Trainium2 Bass kernel for nn_LocalEncoder (4-block local-attention encoder).
#
# Sharding: data-parallel over batch. Core c processes batch element c
# (B=8 == n_cores=8). Same SPMD program on every core, different x slice.
#
# Per-core dataflow: residual x [4096, 256] fp32 lives in SBUF for all 4
# blocks; block weights are DMA'd per block (double buffered); attention is
# computed windowed (128-token windows, look-around of +-1 window) with the
# score matrix built TRANSPOSED (keys on partitions) so A^T feeds the A@V
# matmul directly; softmax denominators come from ones-matmuls on the PE and
# are broadcast back over head rows with a selector matmul.
#
# v2 changes vs baseline (trace-driven):
#  - LN xhat op moved GpSimd -> DVE (gpsimd dispatch was ~4us/instr, 1ms total)
#  - softmax denominator reciprocal via DVE reciprocal_approx_fast instead of
#    ACT Ln+Exp (which forced 3 activation-table reloads per window, 526us)
#  - LN rsqrt computed once per phase for all 32 windows with a DVE
#    Newton-Raphson rsqrt (no ACT table switches left except Gelu)
#  - QKV bias adds dropped (LN beta is zero => folded bias is zero; asserted
#    host-side), PSUM->SBUF moves become plain copies
#  - transpose PSUM->SBUF copies merged to [128,256]
#  - FF w2 output batched per 2 windows, single residual add

import numpy as np
import ml_dtypes

import concourse.bass as bass
import concourse.tile as tile
from concourse import bacc, mybir
from concourse.bass_utils import run_bass_kernel_spmd

F32 = mybir.dt.float32
BF16 = mybir.dt.bfloat16
I32 = mybir.dt.int32
NPBF = ml_dtypes.bfloat16

B, N, D = 8, 4096, 256
H, DH, WIN = 8, 32, 128
NW = N // WIN            # 32 windows
NB = 4                   # encoder blocks
FFI = 682                # geglu inner
FFP = 768                # padded inner (6 k-tiles of 128)
SCALE = DH ** -0.5
T512 = 512               # token tile for dense matmuls
NT = N // T512           # 8 token tiles
EPS = 1e-5
RSQRT_MAGIC = 0x5F3759DF


# ---------------------------------------------------------------- host prep
def _prep_block_weights(i, ln1_g, ln1_b, qkv_w, out_w, ln2_g, ln2_b, ff_w1, ff_w2):
    """Fold LN gamma/beta + softmax scale into weights; pad FF; cast bf16."""
    g1, b1 = ln1_g[i].astype(np.float64), ln1_b[i].astype(np.float64)
    g2, b2 = ln2_g[i].astype(np.float64), ln2_b[i].astype(np.float64)
    Wqkv = qkv_w[i].astype(np.float64)          # [768, 256] (e, d)
    Wg = Wqkv * g1[None, :]
    bias_qkv = Wqkv @ b1                        # [768]
    assert np.allclose(bias_qkv, 0.0), "nonzero QKV bias unsupported"
    # fold softmax scale into Q rows
    Wg[:256] *= SCALE
    wqkT = np.ascontiguousarray(Wg[:512].T)     # [256, 512]
    wvT = np.ascontiguousarray(Wg[512:768].T)   # [256, 256]
    woT = np.ascontiguousarray(out_w[i].astype(np.float64).T)  # [256 e, 256 d]

    W1 = ff_w1[i].astype(np.float64) * g2[None, :]   # [1364, 256]
    b1f = ff_w1[i].astype(np.float64) @ b2           # [1364]
    a_part, g_part = W1[:FFI], W1[FFI:]
    ba, bg = b1f[:FFI], b1f[FFI:]
    assert np.allclose(ba, 0.0), "nonzero FF a-bias unsupported"
    aP = np.zeros((FFP, 256)); aP[:FFI] = a_part
    gP = np.zeros((FFP, 256)); gP[:FFI] = g_part
    bgP = np.zeros((FFP,)); bgP[:FFI] = bg
    w1aT = np.ascontiguousarray(aP.T)            # [256, 768]
    w1gT = np.ascontiguousarray(gP.T)            # [256, 768]
    W2 = np.zeros((FFP, 256)); W2[:FFI] = ff_w2[i].astype(np.float64).T
    w2T = np.ascontiguousarray(W2)               # [768, 256]

    c = lambda a: np.ascontiguousarray(a).astype(NPBF)
    return {
        f"wqkT_{i}": c(wqkT),
        f"wvT_{i}": c(wvT), f"woT_{i}": c(woT),
        f"w1aT_{i}": c(w1aT), f"w1gT_{i}": c(w1gT),
        f"bg_{i}": np.ascontiguousarray(bgP).astype(np.float32),
        f"w2T_{i}": c(w2T),
    }


def _consts():
    ident = np.eye(128, dtype=NPBF)
    ones = np.ones((128, 32), dtype=NPBF)
    return {"ident": ident, "ones1": ones}


# ---------------------------------------------------------------- device IR
def _build(nc):
    """Emit the whole 4-block encoder as one Tile program."""
    x_d = nc.dram_tensor("x", (N, D), F32, kind="ExternalInput").ap()
    out_d = nc.dram_tensor("out", (N, D), F32, kind="ExternalOutput").ap()
    ident_d = nc.dram_tensor("ident", (128, 128), BF16, kind="ExternalInput").ap()
    ones_d = nc.dram_tensor("ones1", (128, 32), BF16, kind="ExternalInput").ap()
    wd = {}
    for i in range(NB):
        wd[f"wqkT_{i}"] = nc.dram_tensor(f"wqkT_{i}", (256, 512), BF16, kind="ExternalInput").ap()
        wd[f"wvT_{i}"] = nc.dram_tensor(f"wvT_{i}", (256, 256), BF16, kind="ExternalInput").ap()
        wd[f"woT_{i}"] = nc.dram_tensor(f"woT_{i}", (256, 256), BF16, kind="ExternalInput").ap()
        wd[f"w1aT_{i}"] = nc.dram_tensor(f"w1aT_{i}", (256, FFP), BF16, kind="ExternalInput").ap()
        wd[f"w1gT_{i}"] = nc.dram_tensor(f"w1gT_{i}", (256, FFP), BF16, kind="ExternalInput").ap()
        wd[f"bg_{i}"] = nc.dram_tensor(f"bg_{i}", (FFP,), F32, kind="ExternalInput").ap()
        wd[f"w2T_{i}"] = nc.dram_tensor(f"w2T_{i}", (FFP, 256), BF16, kind="ExternalInput").ap()

    with tile.TileContext(nc) as tc:
        _emit(tc, x_d, out_d, ident_d, ones_d, wd)
    return nc


def _emit(tc, x_d, out_d, ident_d, ones_d, wd):
    nc = tc.nc
    from contextlib import ExitStack
    ctx = ExitStack()
    with ctx:
        consts = ctx.enter_context(tc.tile_pool(name="consts", bufs=1))
        resid = ctx.enter_context(tc.tile_pool(name="resid", bufs=1))
        seqbuf = ctx.enter_context(tc.tile_pool(name="seqbuf", bufs=1))
        wpool = ctx.enter_context(tc.tile_pool(name="wpool", bufs=2))

        ident = consts.tile([128, 128], BF16)
        nc.sync.dma_start(out=ident, in_=ident_d)
        ones1 = consts.tile([128, 32], BF16)
        nc.sync.dma_start(out=ones1, in_=ones_d)
        # rsqrt seed constant tile (int32 magic), full [128, NW] for TT use
        magic = consts.tile([128, NW], I32)
        nc.vector.memset(magic, RSQRT_MAGIC)

        # residual x, token-major: [128 tok-in-window, 32 windows, 256]
        x_sb = resid.tile([128, NW, D], F32)
        x_wpd = x_d.rearrange("(w p) d -> p w d", p=WIN)
        for c in range(8):
            nc.sync.dma_start(out=x_sb[:, 4 * c:4 * c + 4, :], in_=x_wpd[:, 4 * c:4 * c + 4, :])

        # whole-sequence activation buffers
        qT = seqbuf.tile([128, 2, N], BF16)       # Q^T  rows: g half, (hh*32+dh)
        kT = seqbuf.tile([128, 2, N], BF16)       # K^T
        v_sb = seqbuf.tile([128, NW, H, DH], BF16)  # V token-major
        at_sb = seqbuf.tile([128, 4, H, 3 * WIN], BF16)  # A^T ring (4 slots)

        for blk in range(NB):
            wqk = wpool.tile([128, 2, 512], BF16)
            nc.sync.dma_start(out=wqk, in_=wd[f"wqkT_{blk}"].rearrange("(k p) e -> p k e", p=128))
            wv = wpool.tile([128, 2, 256], BF16)
            nc.sync.dma_start(out=wv, in_=wd[f"wvT_{blk}"].rearrange("(k p) e -> p k e", p=128))
            wo = wpool.tile([128, 2, 256], BF16)
            nc.sync.dma_start(out=wo, in_=wd[f"woT_{blk}"].rearrange("(k p) e -> p k e", p=128))
            w1a = wpool.tile([128, 2, FFP], BF16)
            nc.sync.dma_start(out=w1a, in_=wd[f"w1aT_{blk}"].rearrange("(k p) e -> p k e", p=128))
            w1g = wpool.tile([128, 2, FFP], BF16)
            nc.sync.dma_start(out=w1g, in_=wd[f"w1gT_{blk}"].rearrange("(k p) e -> p k e", p=128))
            bgt = wpool.tile([128, 6], F32)
            nc.sync.dma_start(out=bgt, in_=wd[f"bg_{blk}"].rearrange("(e p) -> p e", p=128))
            w2 = wpool.tile([128, 6, 256], BF16)
            nc.sync.dma_start(out=w2, in_=wd[f"w2T_{blk}"].rearrange("(k p) d -> p k d", p=128))

            _phase_qkv(tc, ctx, x_sb, qT, kT, v_sb, wqk, wv, ident, magic)
            _phase_attn(tc, ctx, x_sb, qT, kT, v_sb, at_sb, wo, ones1)
            _phase_ff(tc, ctx, x_sb, w1a, w1g, bgt, w2, ident, magic)

        out_wpd = out_d.rearrange("(w p) d -> p w d", p=WIN)
        for c in range(8):
            nc.sync.dma_start(out=out_wpd[:, 4 * c:4 * c + 4, :], in_=x_sb[:, 4 * c:4 * c + 4, :])


def _ln_phase_stats(tc, stat, x_sb, magic):
    """LN stats for all NW windows: mv [128, NW, 2] (mean, var) and
    rs [128, NW] = 1/sqrt(var + eps), all on DVE (no ACT)."""
    nc = tc.nc
    alu = mybir.AluOpType
    st = stat.tile([128, NW, 6], F32, tag="st")
    for w in range(NW):
        nc.vector.bn_stats(out=st[:, w, :], in_=x_sb[:, w, :])
    mv = stat.tile([128, NW, 2], F32, tag="mv")
    for w in range(NW):
        nc.vector.bn_aggr(out=mv[:, w, :], in_=st[:, w, :])
    # rs = rsqrt(var + eps): bit-trick seed + 2 Newton iterations, all DVE
    ve = stat.tile([128, NW], F32, tag="ve")
    nc.vector.tensor_scalar(out=ve, in0=mv[:, :, 1], scalar1=EPS, scalar2=None,
                            op0=alu.add)
    sh = stat.tile([128, NW], I32, tag="sh")
    nc.vector.tensor_scalar(out=sh, in0=ve.bitcast(I32), scalar1=1, scalar2=None,
                            op0=alu.logical_shift_right)
    yi = stat.tile([128, NW], I32, tag="yi")
    nc.vector.tensor_tensor(out=yi, in0=magic, in1=sh, op=alu.subtract)
    y = yi.bitcast(F32)
    h = stat.tile([128, NW], F32, tag="h")
    rs = stat.tile([128, NW], F32, tag="rs")
    for it in range(2):
        dst = y if it == 0 else rs
        nc.vector.tensor_tensor(out=h, in0=y, in1=y, op=alu.mult)
        nc.vector.tensor_tensor(out=h, in0=h, in1=ve, op=alu.mult)
        nc.vector.tensor_scalar(out=h, in0=h, scalar1=-0.5, scalar2=1.5,
                                op0=alu.mult, op1=alu.add)
        nc.vector.tensor_tensor(out=dst, in0=y, in1=h, op=alu.mult)
    return mv, rs


def _xhat_t512(tc, pools, x_sb, mv, rs, t, ident, xhT):
    """Normalize + transpose one 512-token tile -> xhat^T bf16 [128, 2, 512]."""
    nc = tc.nc
    stat, ptrans = pools
    alu = mybir.AluOpType
    for q in range(4):
        w = 4 * t + q
        xh = stat.tile([128, D], BF16, tag="xh")
        nc.vector.tensor_scalar(out=xh, in0=x_sb[:, w, :],
                                scalar1=mv[:, w, 0:1], scalar2=rs[:, w:w + 1],
                                op0=alu.subtract, op1=alu.mult)
        pt = ptrans.tile([128, 2, 128], BF16, space="PSUM")
        for dt in range(2):
            nc.tensor.transpose(pt[:, dt, :], xh[:, 128 * dt:128 * dt + 128], ident)
        nc.vector.tensor_copy(out=xhT[:, :, 128 * q:128 * q + 128], in_=pt)


def _phase_qkv(tc, ctx, x_sb, qT, kT, v_sb, wqk, wv, ident, magic):
    nc = tc.nc
    from contextlib import ExitStack
    with ExitStack() as pctx:
        stat = pctx.enter_context(tc.tile_pool(name="stat", bufs=3))
        xhp = pctx.enter_context(tc.tile_pool(name="xhp", bufs=2))
        ptrans = pctx.enter_context(tc.tile_pool(name="ptrans", bufs=2, space="PSUM"))
        mm = pctx.enter_context(tc.tile_pool(name="mmqkv", bufs=3, space="PSUM"))

        mv, rs = _ln_phase_stats(tc, stat, x_sb, magic)
        for t in range(NT):
            xhT = xhp.tile([128, 2, T512], BF16)
            _xhat_t512(tc, (stat, ptrans), x_sb, mv, rs, t, ident, xhT)
            # Q^T / K^T : feature-major [e-tile 128, 512 tok]
            for et in range(4):
                ps = mm.tile([128, T512], F32, space="PSUM")
                for kt in range(2):
                    nc.tensor.matmul(ps, lhsT=wqk[:, kt, 128 * et:128 * et + 128],
                                     rhs=xhT[:, kt, :], start=(kt == 0), stop=(kt == 1))
                dst = qT if et < 2 else kT
                g = et % 2
                nc.vector.tensor_copy(out=dst[:, g, T512 * t:T512 * (t + 1)], in_=ps)
            # V token-major
            for q in range(4):
                w = 4 * t + q
                psv = mm.tile([128, D], F32, space="PSUM", tag="psv", bufs=2)
                for kt in range(2):
                    nc.tensor.matmul(psv, lhsT=xhT[:, kt, 128 * q:128 * q + 128],
                                     rhs=wv[:, kt, :], start=(kt == 0), stop=(kt == 1))
                nc.vector.tensor_copy(out=v_sb[:, w, :, :].rearrange("p h e -> p (h e)"), in_=psv)


def _phase_attn(tc, ctx, x_sb, qT, kT, v_sb, at_sb, wo, ones1):
    nc = tc.nc
    from contextlib import ExitStack
    with ExitStack() as pctx:
        simp = pctx.enter_context(tc.tile_pool(name="simp", bufs=2, space="PSUM"))
        avp = pctx.enter_context(tc.tile_pool(name="avp", bufs=2, space="PSUM"))
        denp = pctx.enter_context(tc.tile_pool(name="denp", bufs=2, space="PSUM"))
        osbp = pctx.enter_context(tc.tile_pool(name="osbp", bufs=3))

        for step in range(NW + 2):
            if step < NW:
                _attn_scores(tc, simp, qT, kT, at_sb, step)
            w = step - 2
            if w >= 0:
                _attn_av(tc, (avp, denp, osbp), x_sb, v_sb, at_sb, wo, ones1, w)


def _attn_scores(tc, simp, qT, kT, at_sb, wp):
    """Block-column pass wp: simT[j in wp, q in wp-1..wp+1] for all heads + exp."""
    nc = tc.nc
    qlo = max(0, wp - 1) * WIN
    qhi = min(NW, wp + 2) * WIN
    qn = qhi - qlo
    aoff = qlo - (wp - 1) * WIN     # column offset inside the 384-wide ring slot
    slot = wp % 4
    for g in range(2):
        for pair in range(2):
            sq = simp.tile([128, 1024], F32, space="PSUM", tag="sim")
            for sub in range(2):
                hh = 2 * pair + sub
                nc.tensor.matmul(
                    sq[:, 512 * sub:512 * sub + qn],
                    lhsT=kT[32 * hh:32 * hh + 32, g, WIN * wp:WIN * (wp + 1)],
                    rhs=qT[32 * hh:32 * hh + 32, g, qlo:qhi],
                    start=True, stop=True, tile_position=(32 * hh, 0))
            src = sq.rearrange("p (s c) -> p s c", c=512)[:, :, 0:qn]
            dst = at_sb[:, slot, 4 * g + 2 * pair:4 * g + 2 * pair + 2, aoff:aoff + qn]
            nc.scalar.activation(out=dst, in_=src, func=mybir.ActivationFunctionType.Exp)


def _attn_av(tc, pools, x_sb, v_sb, at_sb, wo, ones1, w):
    """o_un = A^T-weighted V, denominators, normalize, out-proj, residual."""
    nc = tc.nc
    alu = mybir.AluOpType
    avp, denp, osbp = pools
    wks = [wk for wk in (w - 1, w, w + 1) if 0 <= wk < NW]
    av = avp.tile([128, 256], F32, space="PSUM", tag="av")
    den = denp.tile([128, 256], F32, space="PSUM", tag="den")
    # Loop order: head-group outer, key-window inner. Each col-group's psum
    # accumulation chain completes before the next group's start=True, which
    # is required under both the per-partition (sim) and whole-bank (hw)
    # has_written-clear models.
    for hh in range(4):
        for g in range(2):
            h = 4 * g + hh
            for jt, wk in enumerate(wks):
                slot = wk % 4
                qoff = (w - (wk - 1)) * WIN
                first = g == 0 and jt == 0
                last = jt == len(wks) - 1
                rhs_at = at_sb[:, slot, h, qoff:qoff + WIN]
                nc.tensor.matmul(av[32 * hh:32 * hh + 32, 128 * g:128 * g + 128],
                                 lhsT=v_sb[:, wk, h, :], rhs=rhs_at,
                                 start=first, stop=last, skip_group_check=True,
                                 tile_position=(0, 32 * hh))
    # Denominators: one N=256 matmul per (hh, jt) covers heads hh and hh+4
    # via a strided rhs over the g dim. ones [128, 32] stationary replicates
    # each den over 32 rows -> row-aligned with av for the normalize mul.
    atv = at_sb.rearrange("p s (g hh) q -> p s hh g q", g=2, hh=4)
    for hh in range(4):
        for jt, wk in enumerate(wks):
            slot = wk % 4
            qoff = (w - (wk - 1)) * WIN
            nc.tensor.matmul(den[32 * hh:32 * hh + 32, 0:256],
                             lhsT=ones1, rhs=atv[:, slot, hh, :, qoff:qoff + WIN],
                             start=(jt == 0), stop=(jt == len(wks) - 1),
                             skip_group_check=True, tile_position=(0, 32 * hh))
    # reciprocal on DVE (fast approx, ~18 bits) - keeps ACT exp-only
    rden = osbp.tile([128, 256], F32, tag="rden")
    nc.vector.reciprocal_approx_fast(out=rden, in_=den)
    dp = den  # den bank is dead after the reciprocal; reuse for out-proj delta
    osb = osbp.tile([128, 256], BF16, tag="osb")
    nc.vector.tensor_tensor(out=osb, in0=av, in1=rden, op=alu.mult)
    for g in range(2):
        nc.tensor.matmul(dp, lhsT=osb[:, 128 * g:128 * (g + 1)], rhs=wo[:, g, :],
                         start=(g == 0), stop=(g == 1))
    nc.vector.tensor_tensor(out=x_sb[:, w, :], in0=dp, in1=x_sb[:, w, :],
                            op=alu.add)


def _phase_ff(tc, ctx, x_sb, w1a, w1g, bgt, w2, ident, magic):
    nc = tc.nc
    alu = mybir.AluOpType
    from contextlib import ExitStack
    with ExitStack() as pctx:
        stat = pctx.enter_context(tc.tile_pool(name="statf", bufs=3))
        xhp = pctx.enter_context(tc.tile_pool(name="xhpf", bufs=1))
        ptrans = pctx.enter_context(tc.tile_pool(name="ptransf", bufs=2, space="PSUM"))
        mmf = pctx.enter_context(tc.tile_pool(name="mmf", bufs=2, space="PSUM"))
        ysp = pctx.enter_context(tc.tile_pool(name="ysp", bufs=2))
        glp = pctx.enter_context(tc.tile_pool(name="glp", bufs=2))

        mv, rs = _ln_phase_stats(tc, stat, x_sb, magic)
        xh2 = xhp.tile([128, 2, N], BF16)
        for t in range(NT):
            _xhat_t512(tc, (stat, ptrans), x_sb, mv, rs, t, ident,
                       xh2.rearrange("p k (t c) -> p k t c", c=T512)[:, :, t, :])
        for t in range(NT):
            xhT = xh2[:, :, T512 * t:T512 * (t + 1)]
            ysb = ysp.tile([128, 6, T512], BF16)
            for i in range(6):
                psA = mmf.tile([128, T512], F32, space="PSUM", tag="psA")
                psG = mmf.tile([128, T512], F32, space="PSUM", tag="psG")
                for kt in range(2):
                    nc.tensor.matmul(psA, lhsT=w1a[:, kt, 128 * i:128 * (i + 1)],
                                     rhs=xhT[:, kt, :], start=(kt == 0), stop=(kt == 1))
                for kt in range(2):
                    nc.tensor.matmul(psG, lhsT=w1g[:, kt, 128 * i:128 * (i + 1)],
                                     rhs=xhT[:, kt, :], start=(kt == 0), stop=(kt == 1))
                gl = glp.tile([128, T512], BF16, tag="gl")
                nc.scalar.activation(out=gl, in_=psG, func=mybir.ActivationFunctionType.Gelu,
                                     bias=bgt[:, i:i + 1], scale=1.0)
                nc.vector.tensor_tensor(out=ysb[:, i, :], in0=psA, in1=gl,
                                        op=alu.mult)
            for qq in range(2):
                dp2 = mmf.tile([128, 2, D], F32, space="PSUM", tag="dp2")
                for q2 in range(2):
                    q = 2 * qq + q2
                    for kt in range(6):
                        nc.tensor.matmul(dp2[:, q2, :],
                                         lhsT=ysb[:, kt, 128 * q:128 * (q + 1)],
                                         rhs=w2[:, kt, :], start=(kt == 0), stop=(kt == 5))
                wlo = 4 * t + 2 * qq
                nc.vector.tensor_tensor(out=x_sb[:, wlo:wlo + 2, :], in0=dp2,
                                        in1=x_sb[:, wlo:wlo + 2, :], op=alu.add)


# ---------------------------------------------------------------- entry
_CACHE = {}


def _get_nc():
    if "nc" not in _CACHE:
        nc = bacc.Bacc("TRN2", target_bir_lowering=False, debug=False,
                       enable_asserts=False, num_devices=8)
        _build(nc)
        nc.compile()
        _CACHE["nc"] = nc
    return _CACHE["nc"]


def kernel(x, mask, ln1_g, ln1_b, qkv_w, out_w, ln2_g, ln2_b, ff_w1, ff_w2,
           _trace=False, **kw):
    assert x.shape == (B, N, D)
    nc = _get_nc()
    wmaps = _consts()
    for i in range(NB):
        wmaps.update(_prep_block_weights(i, ln1_g, ln1_b, qkv_w, out_w,
                                         ln2_g, ln2_b, ff_w1, ff_w2))
    in_maps = []
    for c in range(B):
        m = dict(wmaps)
        m["x"] = np.ascontiguousarray(x[c]).astype(np.float32)
        in_maps.append(m)
    res = run_bass_kernel_spmd(nc, in_maps, core_ids=list(range(8)), trace=_trace)
    out = np.stack([res.results[c]["out"] for c in range(B)], axis=0)
    if _trace:
        return out.astype(np.float32), res
    return out.astype(np.float32)
